# revision 44
# baseline (speedup 1.0000x reference)
"""Trainium2 Bass kernel for CayleyStringPE (RoPE + Cayley orthogonal mix).

Math: out = C @ rope(x) per token, where C = (I-S)(I+S)^{-1} is a fixed
128x128 orthogonal matrix (Cayley transform of the skew-symmetric S built
from s_params), and rope applies interleaved-pair rotations by angle
pos[t]*freqs[i].

Device formulation: rope(x)_t = x_t*c_t + P x_t * s_t with P the fixed
pair-swap-sign matrix and c_t/s_t the duplicated cos/sin vectors, so

    out_t = A @ (x_t * c_t) + Bm @ (x_t * s_t),   A = C,  Bm = C @ P

i.e. two 128x128 matmuls per token tile plus two elementwise multiplies.
No cross-partition shuffles on device.

Precision: fp16 end-to-end (inputs, trig tables, weights, outputs) with
f32 PSUM accumulation. fp16's 11-bit mantissa keeps the overall relative
error at ~3e-4 while unlocking the DVE 2x tensor-tensor mode, FWL fast
weight loads, and half the HBM traffic of f32.

Sharding: sequence-parallel across 8 cores (positions split 8 x 1024, all
batches on every core). cos/sin tables are per-core (128 x 1024) and reused
across the 8 batches. A/Bm replicated. No collectives.

Layout: tokens on the SBUF free axis, D=128 on partitions. Host pre-
transposes shards to (128, B*1024) D-major so all DMAs are contiguous.
"""

import sys

import numpy as np

for _p in ("/opt/trn_rl_repo", "/opt/pypackages"):
    if _p not in sys.path:
        sys.path.insert(0, _p)

B, N, D = 8, 8192, 128
NCORES = 8
NSH = N // NCORES          # positions per core
TOK = B * NSH              # tokens per core
CHUNK = 4096               # fused-stream columns per steady DMA chunk (1 MiB fp16)
MMN = 512                  # matmul moving free dim (one PSUM bank, f32)

_NC_CACHE = {}


def _build_nc():
    import concourse.bacc as bacc
    import concourse.mybir as mybir
    import concourse.tile as tile

    f16 = mybir.dt.float16
    f32 = mybir.dt.float32

    nc = bacc.Bacc()
    # q and k undergo the same transform, and the trig tables repeat with
    # period NSH along the token axis, so process ONE fused stream
    # xin = [qT | kT] of 2*TOK columns.
    FTOK = 2 * TOK
    # tbl = [cos (NSH) | sin (NSH) | A (D) | B (D)] fused into one DMA
    TBL = 2 * NSH + 2 * D
    xin = nc.declare_dram_parameter("xin", [D, FTOK], f16, isOutput=False)
    tbl = nc.declare_dram_parameter("tbl", [D, TBL], f16, isOutput=False)
    out = nc.declare_dram_parameter("out", [D, FTOK], f16, isOutput=True)

    # ramped chunk schedule: small first chunks prime the pipeline while the
    # DMAs for the big ones stream in behind them; small last chunks shrink
    # the final out-DMA tail
    sizes = [512, 512, 1024] + [2048] * 6 + [1024, 512, 512]
    assert sum(sizes) == FTOK
    chunks = []
    off = 0
    for s in sizes:
        chunks.append((off, s))
        off += s

    with tile.TileContext(nc) as tc:
        with (
            tc.tile_pool(name="consts", bufs=1) as consts,
            tc.tile_pool(name="inp", bufs=4) as inp,
            tc.tile_pool(name="scaled", bufs=3) as sc,
            tc.tile_pool(name="outp", bufs=3) as outp,
            tc.tile_pool(name="pp", bufs=4, space="PSUM") as pp,
        ):
            tbl_t = consts.tile([D, TBL], f16, tag="tbl", name="tbl_t")
            nc.sync.dma_start(out=tbl_t, in_=tbl[:, :])
            cos_t = tbl_t[:, 0:NSH]
            sin_t = tbl_t[:, NSH : 2 * NSH]
            a_t = tbl_t[:, 2 * NSH : 2 * NSH + D]
            b_t = tbl_t[:, 2 * NSH + D : 2 * NSH + 2 * D]

            PS = 2 * MMN  # up-to-1024-col psum tiles (2 banks); 4 bufs = 8 banks
            n_copies = 0

            for off, size in chunks:
                csl = slice(off, off + size)
                x = inp.tile([D, size], f16, tag="x", name="x")
                nc.sync.dma_start(out=x, in_=xin[:, csl])

                xc = sc.tile([D, size], f16, tag="xc", name="xc")
                xs = sc.tile([D, size], f16, tag="xs", name="xs")
                if size <= NSH:
                    tsl = slice(off % NSH, off % NSH + size)
                    nc.vector.tensor_mul(xc, x, cos_t[:, tsl])
                    nc.vector.tensor_mul(xs, x, sin_t[:, tsl])
                else:
                    r = size // NSH
                    x3 = x.rearrange("p (r n) -> p r n", n=NSH)
                    nc.vector.tensor_mul(
                        xc.rearrange("p (r n) -> p r n", n=NSH),
                        x3,
                        cos_t.unsqueeze(1).broadcast_to((D, r, NSH)),
                    )
                    nc.vector.tensor_mul(
                        xs.rearrange("p (r n) -> p r n", n=NSH),
                        x3,
                        sin_t.unsqueeze(1).broadcast_to((D, r, NSH)),
                    )

                ot = outp.tile([D, size], f16, tag="ot", name="ot")
                pos = 0
                while pos < size:
                    # group up to 2 psum tiles per weight pass: runs of 4
                    # same-weight matmuls pipeline back-to-back on the PE
                    # (a weight reload between matmuls exposes the ~173 ns
                    # PE-SBUF access latency)
                    group = []
                    for _g in range(2):
                        if pos >= size:
                            break
                        ps_w = min(PS, size - pos)
                        ps = pp.tile([D, ps_w], f32, tag="ps", name="ps")
                        group.append((ps, pos, ps_w))
                        pos += ps_w
                    for wt, first in ((a_t, True), (b_t, False)):
                        for ps, p0, w in group:
                            for h in range(w // MMN):
                                sl = slice(p0 + h * MMN, p0 + (h + 1) * MMN)
                                psl = slice(h * MMN, (h + 1) * MMN)
                                src_t = xc if first else xs
                                nc.tensor.matmul(
                                    ps[:, psl], wt, src_t[:, sl],
                                    start=first, stop=not first,
                                )
                    for ps, p0, w in group:
                        osl = slice(p0, p0 + w)
                        # 2:1 ACT/DVE copy split keeps the PSUM drain rate
                        # above the PE production rate
                        if n_copies % 3 == 2:
                            nc.vector.tensor_copy(out=ot[:, osl], in_=ps)
                        else:
                            nc.scalar.copy(out=ot[:, osl], in_=ps)
                        n_copies += 1
                nc.sync.dma_start(out=out[:, csl], in_=ot)

    nc.finalize()
    return nc


def _get_nc():
    if "nc" not in _NC_CACHE:
        _NC_CACHE["nc"] = _build_nc()
    return _NC_CACHE["nc"]


def _default_freqs():
    # computed in f32 end-to-end to match the reference's jnp arithmetic
    e = np.arange(0, D, 2, dtype=np.float32) / np.float32(D)
    return (np.float32(1.0) / np.float32(10000.0) ** e).astype(np.float32)


def _default_s_params():
    # Reproduce reference.setup_inputs()'s jax PRNG stream for s_params.
    # Must run on the CPU backend: the neuron/axon lowering of the threefry
    # PRNG produces a different stream than the CPU one the reference uses.
    import jax

    cpu = jax.local_devices(backend="cpu")[0]
    with jax.default_device(cpu):
        key = jax.random.key(0)
        _, _, k3 = jax.random.split(key, 3)
        num_s = D * (D - 1) // 2
        return np.asarray(
            0.02 * jax.random.normal(k3, (num_s,), dtype="float32"),
            dtype=np.float32,
        )


def _host_prep(pos, freqs, s_params):
    """Cayley matrices (A, Bm as lhsT) and cos/sin tables, all fp16."""
    rows, cols = np.triu_indices(D, 1)
    S = np.zeros((D, D), np.float64)
    sp = np.asarray(s_params, dtype=np.float64)
    S[rows, cols] = sp
    S[cols, rows] = -sp
    I = np.eye(D)
    C = (I - S) @ np.linalg.inv(I + S)
    Bm = np.empty_like(C)
    Bm[:, 0::2] = C[:, 1::2]
    Bm[:, 1::2] = -C[:, 0::2]
    a_lhsT = np.ascontiguousarray(C.T.astype(np.float16))
    b_lhsT = np.ascontiguousarray(Bm.T.astype(np.float16))

    # angle computed in f32 to match the reference's rounding, trig in f64
    ang = np.asarray(freqs, np.float32)[:, None] * np.asarray(pos, np.float32)[None, :]
    ang64 = ang.astype(np.float64)
    cosT = np.repeat(np.cos(ang64), 2, axis=0).astype(np.float16)  # (D, N)
    sinT = np.repeat(np.sin(ang64), 2, axis=0).astype(np.float16)
    return a_lhsT, b_lhsT, cosT, sinT


LAST_RESULTS = None


def kernel(q, k, pos=None, freqs=None, s_params=None, _run_kwargs=None, **_ignored):
    q = np.asarray(q, dtype=np.float32)
    k = np.asarray(k, dtype=np.float32)
    if pos is None:
        pos = np.arange(N, dtype=np.float32)
    if freqs is None:
        freqs = _default_freqs()
    if s_params is None:
        s_params = _default_s_params()

    a_lhsT, b_lhsT, cosT, sinT = _host_prep(pos, freqs, s_params)

    q16 = q.astype(np.float16)
    k16 = k.astype(np.float16)

    in_maps = []
    for c in range(NCORES):
        ssl = slice(c * NSH, (c + 1) * NSH)
        qT = q16[:, ssl, :].reshape(TOK, D).T
        kT = k16[:, ssl, :].reshape(TOK, D).T
        blob = np.concatenate(
            [cosT[:, ssl], sinT[:, ssl], a_lhsT, b_lhsT], axis=1
        )
        in_maps.append(
            {
                "xin": np.ascontiguousarray(np.concatenate([qT, kT], axis=1)),
                "tbl": np.ascontiguousarray(blob),
            }
        )

    from concourse.bass_utils import run_bass_kernel_spmd

    nc = _get_nc()
    res = run_bass_kernel_spmd(
        nc,
        in_maps,
        core_ids=list(range(NCORES)),
        **(_run_kwargs or {}),
    )
    global LAST_RESULTS
    LAST_RESULTS = res

    q_out = np.empty((B, N, D), np.float32)
    k_out = np.empty((B, N, D), np.float32)
    for c in range(NCORES):
        ssl = slice(c * NSH, (c + 1) * NSH)
        o = res.results[c]["out"]
        q_out[:, ssl, :] = o[:, :TOK].T.reshape(B, NSH, D).astype(np.float32)
        k_out[:, ssl, :] = o[:, TOK:].T.reshape(B, NSH, D).astype(np.float32)
    return q_out, k_out


# revision 45
# speedup vs baseline: 1.1522x; 1.1522x over previous
"""Trainium2 Bass kernel for CayleyStringPE (RoPE + Cayley orthogonal mix).

Math: out = C @ rope(x) per token, where C = (I-S)(I+S)^{-1} is a fixed
128x128 orthogonal matrix (Cayley transform of the skew-symmetric S built
from s_params), and rope applies interleaved-pair rotations by angle
pos[t]*freqs[i].

Device formulation: rope(x)_t = x_t*c_t + P x_t * s_t with P the fixed
pair-swap-sign matrix and c_t/s_t the duplicated cos/sin vectors, so

    out_t = A @ (x_t * c_t) + Bm @ (x_t * s_t),   A = C,  Bm = C @ P

i.e. two 128x128 matmuls per token tile plus two elementwise multiplies.
No cross-partition shuffles on device.

Precision: fp16 end-to-end (inputs, trig tables, weights, outputs) with
f32 PSUM accumulation. fp16's 11-bit mantissa keeps the overall relative
error at ~3e-4 while unlocking the DVE 2x tensor-tensor mode, FWL fast
weight loads, and half the HBM traffic of f32.

Sharding: sequence-parallel across 8 cores (positions split 8 x 1024, all
batches on every core). cos/sin tables are per-core (128 x 1024) and reused
across the 8 batches. A/Bm replicated. No collectives.

Layout: tokens on the SBUF free axis, D=128 on partitions. Host pre-
transposes shards to (128, B*1024) D-major so all DMAs are contiguous.
"""

import sys

import numpy as np

for _p in ("/opt/trn_rl_repo", "/opt/pypackages"):
    if _p not in sys.path:
        sys.path.insert(0, _p)

B, N, D = 8, 8192, 128
NCORES = 8
NSH = N // NCORES          # positions per core
TOK = B * NSH              # tokens per core
CHUNK = 4096               # fused-stream columns per steady DMA chunk (1 MiB fp16)
MMN = 512                  # matmul moving free dim (one PSUM bank, f32)

_NC_CACHE = {}


def _build_nc():
    import concourse.bacc as bacc
    import concourse.mybir as mybir
    import concourse.tile as tile

    f16 = mybir.dt.float16
    f32 = mybir.dt.float32

    nc = bacc.Bacc()
    # q and k undergo the same transform, and the trig tables repeat with
    # period NSH along the token axis, so process ONE fused stream
    # xin = [qT | kT] of 2*TOK columns.
    FTOK = 2 * TOK
    # tbl = [cos (NSH) | sin (NSH) | A (D) | B (D)] fused into one DMA
    TBL = 2 * NSH + 2 * D
    xin = nc.declare_dram_parameter("xin", [D, FTOK], f16, isOutput=False)
    tbl = nc.declare_dram_parameter("tbl", [D, TBL], f16, isOutput=False)
    out = nc.declare_dram_parameter("out", [D, FTOK], f16, isOutput=True)

    # ramped chunk schedule: small first chunks prime the pipeline while the
    # DMAs for the big ones stream in behind them; small last chunks shrink
    # the final out-DMA tail
    sizes = [512, 512, 1024] + [2048] * 6 + [1024, 512, 512]
    assert sum(sizes) == FTOK
    chunks = []
    off = 0
    for s in sizes:
        chunks.append((off, s))
        off += s

    with tile.TileContext(nc) as tc:
        with (
            tc.tile_pool(name="consts", bufs=1) as consts,
            tc.tile_pool(name="inp", bufs=4) as inp,
            tc.tile_pool(name="scaled", bufs=3) as sc,
            tc.tile_pool(name="outp", bufs=3) as outp,
            tc.tile_pool(name="pp", bufs=4, space="PSUM") as pp,
        ):
            tbl_t = consts.tile([D, TBL], f16, tag="tbl", name="tbl_t")
            nc.sync.dma_start(out=tbl_t, in_=tbl[:, :])
            cos_t = tbl_t[:, 0:NSH]
            sin_t = tbl_t[:, NSH : 2 * NSH]
            a_t = tbl_t[:, 2 * NSH : 2 * NSH + D]
            b_t = tbl_t[:, 2 * NSH + D : 2 * NSH + 2 * D]

            PS = 2 * MMN  # up-to-1024-col psum tiles (2 banks); 4 bufs = 8 banks
            n_copies = 0

            for off, size in chunks:
                csl = slice(off, off + size)
                x = inp.tile([D, size], f16, tag="x", name="x")
                nc.sync.dma_start(out=x, in_=xin[:, csl])

                xc = sc.tile([D, size], f16, tag="xc", name="xc")
                xs = sc.tile([D, size], f16, tag="xs", name="xs")
                if size <= NSH:
                    tsl = slice(off % NSH, off % NSH + size)
                    nc.vector.tensor_mul(xc, x, cos_t[:, tsl])
                    nc.vector.tensor_mul(xs, x, sin_t[:, tsl])
                else:
                    r = size // NSH
                    x3 = x.rearrange("p (r n) -> p r n", n=NSH)
                    nc.vector.tensor_mul(
                        xc.rearrange("p (r n) -> p r n", n=NSH),
                        x3,
                        cos_t.unsqueeze(1).broadcast_to((D, r, NSH)),
                    )
                    nc.vector.tensor_mul(
                        xs.rearrange("p (r n) -> p r n", n=NSH),
                        x3,
                        sin_t.unsqueeze(1).broadcast_to((D, r, NSH)),
                    )

                ot = outp.tile([D, size], f16, tag="ot", name="ot")
                pos = 0
                while pos < size:
                    ps_w = min(PS, size - pos)
                    osl = slice(pos, pos + ps_w)
                    ps = pp.tile([D, ps_w], f32, tag="ps", name="ps")
                    nh = ps_w // MMN
                    for h in range(nh):
                        sl = slice(pos + h * MMN, pos + (h + 1) * MMN)
                        psl = slice(h * MMN, (h + 1) * MMN)
                        nc.tensor.matmul(
                            ps[:, psl], a_t, xc[:, sl], start=True, stop=False
                        )
                    for h in range(nh):
                        sl = slice(pos + h * MMN, pos + (h + 1) * MMN)
                        psl = slice(h * MMN, (h + 1) * MMN)
                        nc.tensor.matmul(
                            ps[:, psl], b_t, xs[:, sl], start=False, stop=True
                        )
                    # 2:1 ACT/DVE copy split keeps the PSUM drain rate above
                    # the PE production rate without overloading either engine
                    if n_copies % 3 == 2:
                        nc.vector.tensor_copy(out=ot[:, osl], in_=ps)
                    else:
                        nc.scalar.copy(out=ot[:, osl], in_=ps)
                    n_copies += 1
                    pos += ps_w
                nc.sync.dma_start(out=out[:, csl], in_=ot)

    nc.finalize()
    return nc


def _get_nc():
    if "nc" not in _NC_CACHE:
        _NC_CACHE["nc"] = _build_nc()
    return _NC_CACHE["nc"]


def _default_freqs():
    # computed in f32 end-to-end to match the reference's jnp arithmetic
    e = np.arange(0, D, 2, dtype=np.float32) / np.float32(D)
    return (np.float32(1.0) / np.float32(10000.0) ** e).astype(np.float32)


def _default_s_params():
    # Reproduce reference.setup_inputs()'s jax PRNG stream for s_params.
    # Must run on the CPU backend: the neuron/axon lowering of the threefry
    # PRNG produces a different stream than the CPU one the reference uses.
    import jax

    cpu = jax.local_devices(backend="cpu")[0]
    with jax.default_device(cpu):
        key = jax.random.key(0)
        _, _, k3 = jax.random.split(key, 3)
        num_s = D * (D - 1) // 2
        return np.asarray(
            0.02 * jax.random.normal(k3, (num_s,), dtype="float32"),
            dtype=np.float32,
        )


def _host_prep(pos, freqs, s_params):
    """Cayley matrices (A, Bm as lhsT) and cos/sin tables, all fp16."""
    rows, cols = np.triu_indices(D, 1)
    S = np.zeros((D, D), np.float64)
    sp = np.asarray(s_params, dtype=np.float64)
    S[rows, cols] = sp
    S[cols, rows] = -sp
    I = np.eye(D)
    C = (I - S) @ np.linalg.inv(I + S)
    Bm = np.empty_like(C)
    Bm[:, 0::2] = C[:, 1::2]
    Bm[:, 1::2] = -C[:, 0::2]
    a_lhsT = np.ascontiguousarray(C.T.astype(np.float16))
    b_lhsT = np.ascontiguousarray(Bm.T.astype(np.float16))

    # angle computed in f32 to match the reference's rounding, trig in f64
    ang = np.asarray(freqs, np.float32)[:, None] * np.asarray(pos, np.float32)[None, :]
    ang64 = ang.astype(np.float64)
    cosT = np.repeat(np.cos(ang64), 2, axis=0).astype(np.float16)  # (D, N)
    sinT = np.repeat(np.sin(ang64), 2, axis=0).astype(np.float16)
    return a_lhsT, b_lhsT, cosT, sinT


LAST_RESULTS = None


def kernel(q, k, pos=None, freqs=None, s_params=None, _run_kwargs=None, **_ignored):
    q = np.asarray(q, dtype=np.float32)
    k = np.asarray(k, dtype=np.float32)
    if pos is None:
        pos = np.arange(N, dtype=np.float32)
    if freqs is None:
        freqs = _default_freqs()
    if s_params is None:
        s_params = _default_s_params()

    a_lhsT, b_lhsT, cosT, sinT = _host_prep(pos, freqs, s_params)

    q16 = q.astype(np.float16)
    k16 = k.astype(np.float16)

    in_maps = []
    for c in range(NCORES):
        ssl = slice(c * NSH, (c + 1) * NSH)
        qT = q16[:, ssl, :].reshape(TOK, D).T
        kT = k16[:, ssl, :].reshape(TOK, D).T
        blob = np.concatenate(
            [cosT[:, ssl], sinT[:, ssl], a_lhsT, b_lhsT], axis=1
        )
        in_maps.append(
            {
                "xin": np.ascontiguousarray(np.concatenate([qT, kT], axis=1)),
                "tbl": np.ascontiguousarray(blob),
            }
        )

    from concourse.bass_utils import run_bass_kernel_spmd

    nc = _get_nc()
    res = run_bass_kernel_spmd(
        nc,
        in_maps,
        core_ids=list(range(NCORES)),
        **(_run_kwargs or {}),
    )
    global LAST_RESULTS
    LAST_RESULTS = res

    q_out = np.empty((B, N, D), np.float32)
    k_out = np.empty((B, N, D), np.float32)
    for c in range(NCORES):
        ssl = slice(c * NSH, (c + 1) * NSH)
        o = res.results[c]["out"]
        q_out[:, ssl, :] = o[:, :TOK].T.reshape(B, NSH, D).astype(np.float32)
        k_out[:, ssl, :] = o[:, TOK:].T.reshape(B, NSH, D).astype(np.float32)
    return q_out, k_out
